# revision 1
# baseline (speedup 1.0000x reference)
"""Trainium2 kernel for nn_CosinePairwiseLoss.

Math: for unit-normalized rows f_i and class labels pred_i, the reference
computes   loss = 1 - mean_c [ (sum_{i<j, both in c} f_i.f_j) / C(n_c,2) ].
Since sum_{i!=j in c} f_i.f_j = ||S_c||^2 - n_c with S_c = sum_{i in c} f_i,
the strict-lower-triangle sum is (||S_c||^2 - n_c)/2.  So the whole problem
reduces to a per-class segment-sum of normalized rows (C x D) plus counts —
O(N*D) memory-bound work, no N x N similarity matrix.

Device work (per core, rows sharded 8 ways): DMA its [2048, 64] shard in
chunks; squares+row-sums on ACT (accum_out), onehot(pred) via one broadcast
is_equal on DVE, normalize rows on DVE, and accumulate onehot^T @ f_hat into
PSUM with 16 matmuls (contraction over the 128 partitions, accumulated over
the 16 row groups). Output: the core's partial S [64, 64]. Host sums the 8
partials, adds counts (bincount), and finishes the O(C) scalar math.
"""

import numpy as np

N, D, C = 16384, 64, 64
NCORES = 8
ROWS = N // NCORES  # 2048 rows per core
P = 128             # SBUF partitions
NT = ROWS // P      # 16 row groups per partition

# kernel configuration knobs (tuned via TimelineSim)
CFG = {
    "chunks": 2,        # split the shard DMA/compute into this many chunks
    "bf16": True,       # data dtype for feature/onehot/matmul
    "dma_alt": False,   # alternate chunk DMA issue between SP and ACT HWDGE
    "oh_batched": True, # one broadcast is_equal vs per-row-group tensor_scalar
    "sq_act": False,    # squares on ACT via accum_out vs DVE mult+reduce
    "psum_dma": False,  # DMA the PSUM accumulator straight to DRAM
}

_NC_CACHE = {}


def _build_nc(cfg=None):
    import concourse.mybir as mybir
    import concourse.tile as tile
    from concourse import bacc

    cfg = dict(CFG if cfg is None else cfg)
    CH = cfg["chunks"]
    NPC = NT // CH
    f32 = mybir.dt.float32
    dt_data = mybir.dt.bfloat16 if cfg["bf16"] else f32
    Alu = mybir.AluOpType
    Act = mybir.ActivationFunctionType

    nc = bacc.Bacc("TRN2", target_bir_lowering=False, debug=False)

    # combined input: per partition, [pred(NT) | feature(NT*D)] in bf16
    comb_d = nc.dram_tensor("comb", [P, NT + NT * D], dt_data, kind="ExternalInput")
    out_d = nc.dram_tensor("out", [C, D], f32, kind="ExternalOutput")

    with tile.TileContext(nc) as tc:
        with (
            tc.tile_pool(name="const", bufs=1) as const,
            tc.tile_pool(name="fp", bufs=CH) as fpool,
            tc.tile_pool(name="fn", bufs=2) as fnp,
            tc.tile_pool(name="scr", bufs=2) as scrp,
            tc.tile_pool(name="st", bufs=2) as stp,
            tc.tile_pool(name="oh", bufs=4) as ohp,
            tc.tile_pool(name="ps", bufs=1, space="PSUM") as ps,
        ):
            # Dummy sqrt FIRST: forces the act-table pass to pick the sqrt
            # set (which also contains Square/Copy) so the ~1.3us table load
            # overlaps the DMA and never reloads. Kept live via the dbg
            # output (host ignores it).
            zc = const.tile([C, 1], f32)
            nc.gpsimd.memset(zc[:], 0.0)
            dsq = const.tile([C, 1], f32)
            nc.scalar.sqrt(dsq[:], zc[:])

            # DMA order: [pred|chunk0] fused (pred gates the batched onehot,
            # rides along with chunk0), then the remaining chunks; HWDGE
            # issue is serialized (~625ns each).
            comb0 = fpool.tile([P, NT + NPC * D], dt_data, tag="comb0")
            nc.sync.dma_start(comb0[:], comb_d[:, 0 : NT + NPC * D])
            ptile = comb0[:, 0:NT]
            fchs = [comb0[:, NT : NT + NPC * D].rearrange("p (j d) -> p j d", d=D)]
            for k in range(1, CH):
                fch = fpool.tile([P, NPC, D], dt_data, tag="fch")
                dma_eng = nc.scalar if (cfg["dma_alt"] and k % 2) else nc.sync
                dma_eng.dma_start(
                    fch[:],
                    comb_d[:, NT + k * NPC * D : NT + (k + 1) * NPC * D].rearrange(
                        "p (j d) -> p j d", d=D
                    ),
                )
                fchs.append(fch)

            # class-index ramp 0..C-1 (exact in bf16 since C <= 256)
            iot = const.tile([P, C], dt_data)
            nc.gpsimd.iota(
                iot[:], pattern=[[1, C]], base=0, channel_multiplier=0,
                allow_small_or_imprecise_dtypes=True,
            )

            # onehot for all 16 row groups in one broadcast compare:
            # oh[p, n, c] = (iota[p, c] == pred[p, n]) — no norm dependence,
            # so it runs during the feature DMA.
            if cfg["oh_batched"]:
                oh_all = const.tile([P, NT, C], dt_data)
                iotb = iot[:].unsqueeze(1).broadcast_to([P, NT, C])
                predb = ptile.unsqueeze(2).broadcast_to([P, NT, C])
                nc.vector.tensor_tensor(oh_all[:], iotb, predb, Alu.is_equal)

            acc = ps.tile([C, D], f32)
            for k in range(CH):
                fch = fchs[k] if k == 0 else fchs[k][:]
                # sq[p, j] = sum_d fch[p, j, d]^2
                sqch = stp.tile([P, NPC], f32, tag="sq")
                if cfg["sq_act"]:
                    scr = scrp.tile([P, NPC * D], dt_data, tag="scr")
                    for j in range(NPC):
                        nc.scalar.activation(
                            scr[:, j * D : (j + 1) * D],
                            fch[:, j, :],
                            Act.Square,
                            accum_out=sqch[:, j : j + 1],
                        )
                else:
                    scr = scrp.tile([P, NPC, D], dt_data, tag="scr")
                    sq_eng = nc.gpsimd if cfg.get("sq_pool") else nc.vector
                    sq_eng.tensor_mul(scr[:], fch, fch)
                    nc.vector.tensor_reduce(
                        sqch[:], scr[:], axis=mybir.AxisListType.X, op=Alu.add
                    )
                nrmch = stp.tile([P, NPC], f32, tag="nrm")
                nc.scalar.sqrt(nrmch[:], sqch[:])
                rch = stp.tile([P, NPC], f32, tag="rch")
                nc.vector.reciprocal(rch[:], nrmch[:])

                # normalize the chunk: fn[p, j, d] = fch * rnorm[p, j]
                fnch = fnp.tile([P, NPC, D], dt_data, tag="fn")
                rchb = rch[:].unsqueeze(2).broadcast_to([P, NPC, D])
                nc.vector.tensor_tensor(fnch[:], fch, rchb, Alu.mult)

                for j in range(NPC):
                    n = k * NPC + j
                    if cfg["oh_batched"]:
                        oh = oh_all[:, n, :]
                    else:
                        oh_t = ohp.tile([P, C], dt_data, tag="oh")
                        nc.vector.tensor_scalar(
                            oh_t[:], iot[:], ptile[:, n : n + 1], None, Alu.is_equal
                        )
                        oh = oh_t[:]
                    nc.tensor.matmul(
                        acc[:],
                        oh,
                        fnch[:, j, :],
                        start=(n == 0),
                        stop=(n == NT - 1),
                    )

            res = const.tile([C, D], f32)
            # add keeps the dummy sqrt live (dsq is all zeros)
            nc.vector.tensor_scalar(res[:], acc[:], dsq[:], None, Alu.add)
            nc.sync.dma_start(out_d[:], res[:])

    nc.compile()
    return nc


def _get_nc(cfg=None):
    key = "nc" if cfg is None else str(sorted(cfg.items()))
    if key not in _NC_CACHE:
        _NC_CACHE[key] = _build_nc(cfg)
    return _NC_CACHE[key]


def _make_in_maps(feature, pred, cfg=None):
    import ml_dtypes

    cfg = CFG if cfg is None else cfg
    dt_np = ml_dtypes.bfloat16 if cfg["bf16"] else np.float32
    feature = np.asarray(feature).astype(dt_np)
    pred_f = np.asarray(pred).astype(dt_np)
    in_maps = []
    for c in range(NCORES):
        fs = feature[c * ROWS : (c + 1) * ROWS].reshape(P, NT * D)
        ps_ = pred_f[c * ROWS : (c + 1) * ROWS].reshape(P, NT)
        comb = np.ascontiguousarray(np.concatenate([ps_, fs], axis=1))
        in_maps.append({"comb": comb})
    return in_maps


def _finish(partials, pred):
    """Combine per-core partial segment sums into the scalar loss."""
    pred_i = np.asarray(pred).astype(np.int64)
    S = np.zeros((C, D), np.float64)
    for p in partials:
        S += p.astype(np.float64)
    counts = np.bincount(pred_i, minlength=C).astype(np.float64)
    cls_pair_sum = 0.5 * ((S * S).sum(axis=1) - counts)
    pair_counts = counts * (counts - 1.0) * 0.5
    avg = np.where(pair_counts > 0, cls_pair_sum / np.maximum(pair_counts, 1.0), 0.0)
    n_unique = float((counts > 0).sum())
    loss = 1.0 - avg.sum() / n_unique
    return np.float32(loss)


def _run(feature, pred, trace=False, **spmd_kwargs):
    from concourse.bass_utils import run_bass_kernel_spmd

    nc = _get_nc()
    in_maps = _make_in_maps(feature, pred)
    res = run_bass_kernel_spmd(
        nc, in_maps, core_ids=list(range(NCORES)), trace=trace, **spmd_kwargs
    )
    partials = [r["out"] for r in res.results]
    return _finish(partials, pred), res


def kernel(feature, pred, num_classes):
    assert int(num_classes) == C
    loss, _ = _run(feature, pred, trace=False)
    return loss

